# revision 6
# baseline (speedup 1.0000x reference)
"""VQ codebook (cosine-sim argmax) Trainium2 kernel.

Full-input contract: kernel(x=[8,2048,512] f32, codebook=[16384,512] f32)
returns (z_q_ste [8,2048,512] f32, indices [8,2048] int32,
usage_entropy f32 scalar, mean_sim f32 scalar) matching reference.py.

Strategy: data-parallel over the 16384 tokens across 8 NeuronCores
(2048 tokens/core), codebook (active 8192 rows) replicated. Each core
computes cosine sims via PE matmuls (contraction over D=512 on the
partition axis) and reduces each token's 8192 sims to top-8
(value, index) candidates with DVE max/max_index. The host then
rescores the top-8 candidates in float64 to pick the exact argmax and
computes the cheap O(N) epilogue (gather, STE, entropy, mean).
"""

import numpy as np

ACTIVE_V = 8192
D = 512
N_CORES = 8
B, T = 8, 2048
N_TOK = B * T
N_SHARD = N_TOK // N_CORES  # 2048
EPS = 1e-12

# ---------------------------------------------------------------------------
# TileContext tail-drain patch: walrus in this toolchain accepts at most ONE
# embedded sem wait on CTRL-type instructions (NoOp/Drain). Tile's kernel-tail
# drain accumulates one wait per live processor, which trips
# "Too many sync wait commands". Split the waits across a chain of SP NOPs.
# ---------------------------------------------------------------------------


def _apply_tile_patch():
    import bass_rust
    import concourse.mybir as mybir
    import concourse.tile as tile
    from concourse.vector_clock import ScopedClock

    if getattr(tile.TileContext, "_wait_patches_applied", False):
        return

    MAX_WAITS = 1

    def _split_waits(tc, inst):
        si = inst.sync_info
        if si is None:
            return
        waits = list(si.on_wait)
        if len(waits) <= MAX_WAITS:
            return
        if inst.engine == mybir.EngineType.Unassigned:
            return
        updates = list(si.on_update)
        extra, keep = waits[:-MAX_WAITS], waits[-MAX_WAITS:]
        for i in range(0, len(extra), MAX_WAITS):
            nop = mybir.InstNoOp(
                name=tc.nc.get_next_instruction_name(),
                engine=inst.engine,
                ins=[],
                outs=[],
            )
            nop.sync_info = bass_rust.SyncInfo(
                on_wait=extra[i:i + MAX_WAITS], on_update=[]
            )
            tc._add_instruction(nop)
        inst.sync_info = bass_rust.SyncInfo(on_wait=keep, on_update=updates)

    _orig_commit = tile.TileContext._commit_instruction

    def _patched_commit_instruction(self, inst, lazy_reg_writes=True):
        _split_waits(self, inst)
        return _orig_commit(self, inst, lazy_reg_writes)

    def _patched_drain_and_barrier(self, tick_clock, wait_clock):
        nc = self.nc
        nop1 = nc.sync.nop()
        wait_clock.add_sem_waits(
            nop1.ins, ScopedClock({None: tick_clock.global_clock})
        )
        si = nop1.ins.sync_info
        if si is not None and len(si.on_wait) > 1:
            waits = list(si.on_wait)
            updates = list(si.on_update)
            nop1.ins.sync_info = bass_rust.SyncInfo(
                on_wait=waits[:1], on_update=updates
            )
            for w in waits[1:]:
                nopn = nc.sync.nop()
                nopn.ins.sync_info = bass_rust.SyncInfo(on_wait=[w], on_update=[])
        nc.sync.drain()
        nc.all_engine_barrier()
        assert self.sems is not None
        popped = nc._tile_sem_poison_stack.pop()
        assert popped is self._sem_poison
        nc.clear_and_free_semaphores(list(self.sems.allocated().values()))
        nc.all_engine_barrier()

    tile.TileContext._commit_instruction = _patched_commit_instruction
    tile.TileContext._drain_and_barrier = _patched_drain_and_barrier
    tile.TileContext._wait_patches_applied = True


# ---------------------------------------------------------------------------
# Kernel builder
# ---------------------------------------------------------------------------


def build_nc(n_tok=N_SHARD, v_act=ACTIVE_V, use_f32r=False, sims_bf16=False):
    import concourse.bass as bass
    import concourse.mybir as mybir
    import concourse.tile as tile

    _apply_tile_patch()

    f32 = mybir.dt.float32
    u32 = mybir.dt.uint32
    mmdt = f32
    simdt = mybir.dt.bfloat16 if sims_bf16 else f32

    def mm_ap(ap):
        return ap.bitcast(mybir.dt.float32r) if use_f32r else ap

    KC = D // 128  # 4 contraction chunks
    TT = n_tok // 128  # token tiles
    CT = v_act // 512  # code tiles (512 codes each)
    VG = max(1, v_act // 2048)  # codebook column groups for prep pipelining
    vg_codes = v_act // VG  # codes per group

    nc = bass.Bass(debug=False)
    x = nc.dram_tensor("x", [n_tok, D], f32, kind="ExternalInput")
    cb = nc.dram_tensor("cb", [v_act, D], f32, kind="ExternalInput")
    ident = nc.dram_tensor("ident", [128, 128], f32, kind="ExternalInput")
    vals_out = nc.dram_tensor("vals", [128, TT * 8], f32, kind="ExternalOutput")
    idx_out = nc.dram_tensor("idxs", [128, TT * 8], u32, kind="ExternalOutput")

    with tile.TileContext(nc) as tc:
        with (
            tc.tile_pool(name="const", bufs=1) as const_pool,
            tc.tile_pool(name="cbT", bufs=1) as cbT_pool,
            tc.tile_pool(name="outacc", bufs=1) as out_pool,
            tc.tile_pool(name="cbload", bufs=3) as cbload_pool,
            tc.tile_pool(name="cbnorm", bufs=3) as cbnorm_pool,
            tc.tile_pool(name="stats", bufs=4) as stats_pool,
            tc.tile_pool(name="xload", bufs=2) as xload_pool,
            tc.tile_pool(name="xT", bufs=3) as xT_pool,
            tc.tile_pool(name="sims", bufs=1) as sims_pool,
            tc.tile_pool(name="prep_psum", bufs=4, space="PSUM") as prep_psum,
            tc.tile_pool(name="mm_psum", bufs=4, space="PSUM") as mm_psum,
        ):
            ident_sb = const_pool.tile([128, 128], f32)
            nc.sync.dma_start(ident_sb[:], ident[:])

            vals_sb = out_pool.tile([128, TT * 8], f32)
            idx_sb = out_pool.tile([128, TT * 8], u32)

            # --- codebook prep: normalize rows, transpose into [d, v] ---
            # cbT group g holds codes [g*vg_codes, (g+1)*vg_codes), laid out
            # as KC chunks of columns: chunk k at [k*vg_codes, (k+1)*vg_codes)
            cbT_g = [cbT_pool.tile([128, KC * vg_codes], mmdt,
                                   name=f"cbT{g}", tag=f"cbT{g}")
                     for g in range(VG)]
            for vt in range(v_act // 128):
                v0 = vt * 128
                g = v0 // vg_codes
                vloc = v0 - g * vg_codes
                a = cbload_pool.tile([128, D], f32)
                nc.sync.dma_start(a[:], cb[v0:v0 + 128, :])
                sq = cbnorm_pool.tile([128, D], f32, tag="sq")
                ssq = stats_pool.tile([128, 1], f32, tag="ssq")
                nc.scalar.activation(
                    sq[:], a[:], mybir.ActivationFunctionType.Square,
                    accum_out=ssq[:],
                )
                nrm = stats_pool.tile([128, 1], f32, tag="nrm")
                nc.scalar.sqrt(nrm[:], ssq[:])
                rinv = stats_pool.tile([128, 1], f32, tag="rinv")
                nc.vector.reciprocal(rinv[:], nrm[:])
                b = cbnorm_pool.tile([128, D], mmdt, tag="bnorm")
                nc.vector.tensor_scalar_mul(b[:], a[:], rinv[:])
                for k in range(KC):
                    pt = prep_psum.tile([128, 128], f32)
                    nc.tensor.transpose(
                        pt[:], b[:, k * 128:(k + 1) * 128], ident_sb[:]
                    )
                    nc.scalar.copy(
                        cbT_g[g][:, k * vg_codes + vloc:
                                 k * vg_codes + vloc + 128],
                        pt[:],
                    )

            # --- main: per token tile, transpose x then sims matmuls + top-8 ---
            for tt in range(TT):
                t0 = tt * 128
                xa = xload_pool.tile([128, D], f32)
                nc.sync.dma_start(xa[:], x[t0:t0 + 128, :])
                xT = xT_pool.tile([128, KC * 128], mmdt)
                for k in range(KC):
                    pt = prep_psum.tile([128, 128], f32)
                    nc.tensor.transpose(
                        pt[:], xa[:, k * 128:(k + 1) * 128], ident_sb[:]
                    )
                    nc.scalar.copy(xT[:, k * 128:(k + 1) * 128], pt[:])
                sims = sims_pool.tile([128, v_act], simdt)
                for ct in range(CT):
                    c0 = ct * 512
                    g = c0 // vg_codes
                    cloc = c0 - g * vg_codes
                    ps = mm_psum.tile([128, 512], f32)
                    for k in range(KC):
                        nc.tensor.matmul(
                            ps[:],
                            mm_ap(xT[:, k * 128:(k + 1) * 128]),
                            mm_ap(cbT_g[g][:, k * vg_codes + cloc:
                                           k * vg_codes + cloc + 512]),
                            start=(k == 0),
                            stop=(k == KC - 1),
                        )
                    nc.scalar.copy(sims[:, c0:c0 + 512], ps[:])
                v8 = vals_sb[:, tt * 8:(tt + 1) * 8]
                nc.vector.max(v8, sims[:])
                nc.vector.max_index(idx_sb[:, tt * 8:(tt + 1) * 8], v8, sims[:])

            nc.sync.dma_start(vals_out[:], vals_sb[:])
            nc.sync.dma_start(idx_out[:], idx_sb[:])

    return nc


# ---------------------------------------------------------------------------
# Host-side wrapper
# ---------------------------------------------------------------------------

_RUN_CACHE = {}


def _run_device(x_flat, cb_active, trace=False):
    """Run the SPMD kernel; returns (top_vals [N,8] f32, top_idx [N,8] u32,
    exec_time_ns or None)."""
    from concourse.bass_utils import run_bass_kernel_spmd

    key = "nc"
    if key not in _RUN_CACHE:
        _RUN_CACHE[key] = build_nc()
    nc = _RUN_CACHE[key]

    ident = np.eye(128, dtype=np.float32)
    in_maps = []
    for c in range(N_CORES):
        sl = x_flat[c * N_SHARD:(c + 1) * N_SHARD]
        in_maps.append({
            "x": np.ascontiguousarray(sl),
            "cb": cb_active,
            "ident": ident,
        })
    res = run_bass_kernel_spmd(
        nc, in_maps, core_ids=list(range(N_CORES)), trace=False
    )
    TTt = N_SHARD // 128
    tv = np.empty((N_TOK, 8), np.float32)
    ti = np.empty((N_TOK, 8), np.uint32)
    for c in range(N_CORES):
        v = res.results[c]["vals"].reshape(128, TTt, 8).transpose(1, 0, 2)
        i = res.results[c]["idxs"].reshape(128, TTt, 8).transpose(1, 0, 2)
        tv[c * N_SHARD:(c + 1) * N_SHARD] = v.reshape(N_SHARD, 8)
        ti[c * N_SHARD:(c + 1) * N_SHARD] = i.reshape(N_SHARD, 8)
    return tv, ti, res.exec_time_ns


def kernel(x, codebook, _trace=False, _return_exec_ns=False):
    x = np.asarray(x, dtype=np.float32)
    codebook = np.asarray(codebook, dtype=np.float32)
    x_flat = np.ascontiguousarray(x.reshape(-1, D))
    cb_active = np.ascontiguousarray(codebook[:ACTIVE_V])

    top_vals, top_idx, exec_ns = _run_device(x_flat, cb_active, trace=_trace)

    # Host epilogue: exact fp32 normalization (mirrors reference), float64
    # rescore of the top-8 candidates per token -> exact argmax.
    xn = np.sqrt(np.sum(x_flat.astype(np.float64) ** 2, axis=1))
    x_norm = (x_flat / np.maximum(np.sqrt(
        np.sum(x_flat * x_flat, axis=1, keepdims=True)), EPS)).astype(np.float32)
    cbn = np.maximum(np.sqrt(
        np.sum(cb_active * cb_active, axis=1, keepdims=True)), EPS)
    cb_norm = (cb_active / cbn).astype(np.float32)

    cand = top_idx.astype(np.int64)  # [N, 8]
    cand_vecs = cb_norm[cand]  # [N, 8, D]
    d = np.einsum(
        "nd,nkd->nk", x_norm.astype(np.float64), cand_vecs.astype(np.float64)
    )  # [N, 8]
    j = np.argmax(d, axis=1)
    rows = np.arange(d.shape[0])
    indices = cand[rows, j]  # [N]
    max_sims = d[rows, j].astype(np.float32)

    z_q_flat = cb_active[indices]  # [N, D] f32
    z_q_ste = (x_flat + (z_q_flat - x_flat)).reshape(B, T, D)

    counts = np.bincount(indices, minlength=ACTIVE_V).astype(np.float32)
    probs = counts / np.float32(N_TOK + 1e-9)
    nz = probs > 0
    usage_entropy = np.float32(
        -np.sum(np.where(nz, probs * np.log(np.maximum(probs, EPS),
                                            dtype=np.float32), 0.0))
    )
    mean_sim = np.float32(np.mean(max_sims))

    out = (
        z_q_ste.astype(np.float32),
        indices.reshape(B, T).astype(np.int32),
        np.float32(usage_entropy),
        np.float32(mean_sim),
    )
    if _return_exec_ns:
        return out, exec_ns
    return out
